# revision 11
# baseline (speedup 1.0000x reference)
"""Multi-head self-attention on 8 TRN2 NeuronCores — v2.

Sharding: core c -> (batch b = c//2, head-half g = c%2, heads 8g..8g+7).
Each core computes qkv-proj + attention + out-proj partial for its 8 heads;
host sums the two partials per batch and adds the output bias.

Stage 1 (qkv projections) runs as fp8e4m3 DoubleRow matmuls (K=256 per
instruction at 0.5 cyc/row): x and the weights are host-folded to a
[128, 2, 4, *] layout with embed index e = 256*gk + 128*i + p. The q/k
biases are added exactly during the PSUM->SBUF copy as per-partition
tensor_scalar adds; the V bias is folded into the host-side output bias
(softmax rows sum to 1, so ctx = ctx_nobias + b_v, and b_v @ W_out is a
constant added on the host).

Scores run fp16 per head pair (K=64 row-packed via partition bases).
Exp on ACT (the overall bottleneck) writes fp8 probabilities with the two
sk-chunks of each score tile side by side; PV then runs fp8 DoubleRow
with those two chunks as the two K-slots, producing ctx TRANSPOSED
([q, d] with q on all 128 partitions) plus an appended ones-column that
accumulates the softmax denominator per q row. Normalization is a
per-partition reciprocal+scale on DVE, then a PE fp16 transpose restores
[d, q] for the fp16 out-projection.
"""
import sys
sys.path.insert(0, '/opt/trn_rl_repo')

import numpy as np
import ml_dtypes

import concourse.bass as bass
import concourse.mybir as mybir
import concourse.tile as tile
from concourse import bacc

F32 = mybir.dt.float32
F16 = mybir.dt.float16
F8 = mybir.dt.float8e4
E8NP = ml_dtypes.float8_e4m3
DR = mybir.MatmulPerfMode.DoubleRow
EXP = mybir.ActivationFunctionType.Exp

B, S, D = 4, 2048, 1024
H, HD = 16, 64            # total heads, head dim
HC = 8                    # heads per core
N_CORES = 8
SC = S // 512             # seq chunks of 512
NSK = S // 128            # sk chunks of 128
NG = NSK // 2             # DoubleRow groups of 2 sk chunks


def build_nc(debug=False):
    nc = bacc.Bacc(None, target_bir_lowering=False)

    x8 = nc.dram_tensor("x8", [128, 2, 4, S], F8, kind="ExternalInput")
    w_qk8 = nc.dram_tensor("w_qk8", [128, 2, 4, 1024], F8, kind="ExternalInput")
    x16 = nc.dram_tensor("x16", [128, 8, S], F16, kind="ExternalInput")
    w_v16 = nc.dram_tensor("w_v16", [128, 8, 512], F16, kind="ExternalInput")
    b_qk = nc.dram_tensor("b_qk", [128, 4, 2], F32, kind="ExternalInput")
    ident = nc.dram_tensor("ident", [128, 128], F16, kind="ExternalInput")
    w_out = nc.dram_tensor("w_out", [128, 4, 1024], F16, kind="ExternalInput")
    out = nc.dram_tensor("out", [S, D], F32, kind="ExternalOutput")
    if debug:
        dbg_pjt = nc.dram_tensor("dbg_pjt", [128, 2, S], F16,
                                 kind="ExternalOutput")
        dbg_v8 = nc.dram_tensor("dbg_v8", [128, NSK, HC, HD + 1], F16,
                                kind="ExternalOutput")
        dbg_expT = nc.dram_tensor("dbg_expT", [128, NSK, 2, 512], F16,
                                  kind="ExternalOutput")
        dbg_ctxT = nc.dram_tensor("dbg_ctxT", [128, 4, S], F16,
                                  kind="ExternalOutput")

    with tile.TileContext(nc) as tc:
        with (
            tc.tile_pool(name="const", bufs=1) as cpool,
            tc.tile_pool(name="pjt", bufs=2) as pjt_pool,
            tc.tile_pool(name="vpool", bufs=1) as vpool,
            tc.tile_pool(name="expT", bufs=2) as expT_pool,
            tc.tile_pool(name="ctxTp", bufs=1) as ctxT_pool,
            tc.tile_pool(name="ctxs", bufs=3) as ctx_pool,
            tc.tile_pool(name="rcpp", bufs=3) as rcp_pool,
            tc.tile_pool(name="outsb", bufs=2) as out_pool,
        ):
            x8_sb = cpool.tile([128, 2, 4, S], F8)
            nc.sync.dma_start(x8_sb[:], x8[:])
            w_qk_sb = cpool.tile([128, 2, 4, 1024], F8)
            nc.sync.dma_start(w_qk_sb[:], w_qk8[:])
            b_qk_sb = cpool.tile([128, 4, 2], F32)
            nc.sync.dma_start(b_qk_sb[:], b_qk[:])
            ident_sb = cpool.tile([128, 128], F16)
            nc.sync.dma_start(ident_sb[:], ident[:])
            w_out_sb = cpool.tile([128, 4, 1024], F16)
            nc.sync.dma_start(w_out_sb[:], w_out[:])

            # v8[s % 128, s_tile, head, 0:64] = V (no bias); [..., 64] = 1.0
            v16 = vpool.tile([128, NSK, HC, HD + 1], F16)
            nc.vector.memset(v16[:, :, :, HD], 1.0)
            ctxT = ctxT_pool.tile([128, 4, S], F16)

            def s1_pair_chunk(p, pjt, n, pool):
                """q,k projection of pair p for seq chunk n -> pjt[:, :, n]."""
                for j in range(2):     # 0 = q, 1 = k
                    ps = pool.tile([128, 512], F32, name="s1ps", tag="s1op")
                    cs = 512 * j + 128 * p
                    for gk in range(4):
                        nc.tensor.matmul(
                            ps[:], w_qk_sb[:, :, gk, cs:cs + 128],
                            x8_sb[:, :, gk, 512 * n:512 * (n + 1)],
                            start=(gk == 0), stop=(gk == 3), perf_mode=DR)
                    nc.vector.tensor_scalar_add(
                        pjt[:, j, 512 * n:512 * (n + 1)], ps[:],
                        b_qk_sb[:, p, j:j + 1])

            # ---- prologue: pair-0 q,k projection + V projection (all heads)
            # in a multi-bank PSUM pool that closes before attention opens ----
            pjt0 = pjt_pool.tile([128, 2, S], F16, name="pjt", tag="pjt")
            with tc.tile_pool(name="s1wv", bufs=1) as wv_pool, \
                 tc.tile_pool(name="pro_ps", bufs=2, space="PSUM") as pro_ps:
                w_v_sb = wv_pool.tile([128, 8, 512], F16)
                nc.sync.dma_start(w_v_sb[:], w_v16[:])
                x16_sb = cpool.tile([128, 8, S], F16)
                nc.sync.dma_start(x16_sb[:], x16[:])
                for n in range(SC):
                    s1_pair_chunk(0, pjt0, n, pro_ps)
                for n in range(SC):
                    for tl in range(4):
                        t = 4 * n + tl
                        ps = pro_ps.tile([128, 512], F32, name="vps", tag="s1op")
                        for ec in range(8):
                            nc.tensor.matmul(
                                ps[:],
                                x16_sb[:, ec, 128 * t:128 * (t + 1)],
                                w_v_sb[:, ec, :],
                                start=(ec == 0), stop=(ec == 7))
                        nc.vector.tensor_copy(
                            v16[:, t, :, 0:HD],
                            ps.rearrange("p (h d) -> p h d", h=HC))

            # ---- attention ----
            with (
                tc.tile_pool(name="scps", bufs=2, space="PSUM") as sc_ps,
                tc.tile_pool(name="pvps", bufs=2, space="PSUM") as pv_ps,
                tc.tile_pool(name="tpps", bufs=1, space="PSUM") as tp_ps,
                tc.tile_pool(name="s1op", bufs=1, space="PSUM") as s1op_ps,
            ):
                pjts = {0: pjt0}
                if debug:
                    nc.sync.dma_start(dbg_pjt[:], pjt0[:])
                    nc.sync.dma_start(dbg_v8[:], v16[:])
                for p in range(4):
                    pjt = pjts.pop(p)
                    for qc in range(SC):
                        qsl = slice(512 * qc, 512 * (qc + 1))
                        expT = expT_pool.tile([128, NSK, 2, 512], F16,
                                              name="expT", tag="expT")
                        for hi in range(2):
                            base = 64 * hi
                            for grp in range(NG):
                                scp = sc_ps.tile([128, 2, 512], F32,
                                                 name="scp", tag="scp")
                                for gg in range(2):
                                    sk = 2 * grp + gg
                                    nc.tensor.matmul(
                                        scp[:, gg, :],
                                        pjt[base:base + 64, 1,
                                            128 * sk:128 * (sk + 1)],
                                        pjt[base:base + 64, 0, qsl],
                                        start=True, stop=True)
                                nc.scalar.activation(
                                    expT[:, 2 * grp:2 * grp + 2, hi, :],
                                    scp[:], EXP, scale=0.125)

                        if debug and p == 0 and qc == 0:
                            nc.sync.dma_start(dbg_expT[:], expT[:])
                        # overlap next pair's projection under the ACT-bound
                        # attention, one seq chunk per q-chunk
                        if p < 3:
                            if qc == 0:
                                pjts[p + 1] = pjt_pool.tile(
                                    [128, 2, S], F16, name="pjt", tag="pjt")
                            s1_pair_chunk(p + 1, pjts[p + 1], qc, s1op_ps)

                        # PV (fp8 DoubleRow) + normalize + transpose
                        for hi in range(2):
                            h = 2 * p + hi
                            base = 64 * hi
                            for jq in range(4):
                                qb = 512 * qc + 128 * jq
                                pv = pv_ps.tile([128, 512], F32,
                                                name="pv", tag="pv")
                                for sk in range(NSK):
                                    nc.tensor.matmul(
                                        pv[:, 0:HD + 1],
                                        expT[:, sk, hi,
                                             128 * jq:128 * (jq + 1)],
                                        v16[:, sk, h, :],
                                        start=(sk == 0), stop=(sk == NSK - 1))
                                rcp = rcp_pool.tile([128, 1], F32, name="rcp",
                                                    tag="rcp")
                                nc.vector.reciprocal(rcp[:], pv[:, HD:HD + 1])
                                cx = ctx_pool.tile([128, HD], F16, name="cx",
                                                   tag="cx")
                                nc.vector.tensor_scalar_mul(
                                    cx[:], pv[:, 0:HD], rcp[:])
                                tp = tp_ps.tile([64, 1024], F16, name="tp",
                                                tag="tp")
                                nc.tensor.transpose(
                                    tp[:, 0:128], cx[:], ident_sb[:])
                                nc.vector.tensor_copy(
                                    ctxT[base:base + 64, p, qb:qb + 128],
                                    tp[:, 0:128])

                        # out-projection for this q-chunk once the last
                        # pair's ctxT columns are in place
                        if p == 3:
                            for tl in range(4):
                                tq = 4 * qc + tl
                                for ec in range(2):
                                    ps4 = s1op_ps.tile([128, 512], F32,
                                                       name="s4", tag="s1op")
                                    for pp in range(4):
                                        nc.tensor.matmul(
                                            ps4[:],
                                            ctxT[:, pp,
                                                 128 * tq:128 * (tq + 1)],
                                            w_out_sb[:, pp,
                                                     512 * ec:512 * (ec + 1)],
                                            start=(pp == 0), stop=(pp == 3))
                                    o = out_pool.tile([128, 512], F32,
                                                      name="o")
                                    nc.vector.tensor_copy(o[:], ps4[:])
                                    nc.sync.dma_start(
                                        out[128 * tq:128 * (tq + 1),
                                            512 * ec:512 * (ec + 1)], o[:])

                if debug:
                    nc.sync.dma_start(dbg_ctxT[:], ctxT[:])

    nc.compile()
    return nc


# ---------------------------------------------------------------------------
# host side: shard, run SPMD, gather
# ---------------------------------------------------------------------------

_RUNNER = None


def _make_runner(nc, n_cores):
    """Jit-once SPMD runner via PJRT (axon)."""
    import jax
    from jax.sharding import Mesh, PartitionSpec
    from jax.experimental.shard_map import shard_map
    from concourse import bass2jax
    from concourse.bass2jax import _bass_exec_p, install_neuronx_cc_hook

    install_neuronx_cc_hook()
    partition_name = nc.partition_id_tensor.name if nc.partition_id_tensor else None

    in_names, out_names, out_avals, zero_outs = [], [], [], []
    for alloc in nc.m.functions[0].allocations:
        if not isinstance(alloc, mybir.MemoryLocationSet):
            continue
        name = alloc.memorylocations[0].name
        if alloc.kind == "ExternalInput":
            if name != partition_name:
                in_names.append(name)
        elif alloc.kind == "ExternalOutput":
            out_names.append(name)
            shape = tuple(alloc.tensor_shape)
            dtype = mybir.dt.np(alloc.dtype)
            out_avals.append(jax.core.ShapedArray(shape, dtype))
            zero_outs.append(np.zeros(shape, dtype))
    n_params = len(in_names)
    n_outs = len(out_avals)
    all_in_names = list(in_names) + list(out_names)
    if partition_name is not None:
        all_in_names.append(partition_name)

    def _body(*args):
        operands = list(args)
        if partition_name is not None:
            operands.append(bass2jax.partition_id_tensor())
        outs = _bass_exec_p.bind(
            *operands,
            out_avals=tuple(out_avals),
            in_names=tuple(all_in_names),
            out_names=tuple(out_names),
            lowering_input_output_aliases=(),
            sim_require_finite=True,
            sim_require_nnan=True,
            nc=nc,
        )
        return tuple(outs)

    devices = jax.devices()[:n_cores]
    if n_cores == 1:
        jitted = jax.jit(_body, keep_unused=True)

        def run1(in_maps):
            args = [np.asarray(in_maps[0][n]) for n in in_names] + list(zero_outs)
            out_arrs = jitted(*args)
            jax.block_until_ready(out_arrs)
            return [{n: np.asarray(out_arrs[i]) for i, n in enumerate(out_names)}]

        return run1

    mesh = Mesh(np.asarray(devices), ("core",))
    in_specs = (PartitionSpec("core"),) * (n_params + n_outs)
    out_specs = (PartitionSpec("core"),) * n_outs
    jitted = jax.jit(
        shard_map(_body, mesh=mesh, in_specs=in_specs, out_specs=out_specs,
                  check_rep=False),
        keep_unused=True,
    )

    def run(in_maps):
        concat_in = [
            np.concatenate([np.asarray(in_maps[c][n]) for c in range(n_cores)],
                           axis=0)
            for n in in_names
        ]
        concat_zero = [
            np.zeros((n_cores * z.shape[0], *z.shape[1:]), z.dtype)
            for z in zero_outs
        ]
        out_arrs = jitted(*concat_in, *concat_zero)
        jax.block_until_ready(out_arrs)
        return [
            {n: np.asarray(out_arrs[i]).reshape(n_cores, *out_avals[i].shape)[c]
             for i, n in enumerate(out_names)}
            for c in range(n_cores)
        ]

    return run


def _fold(m):
    """[1024, C] -> [128, 2, 4, C] with row e = 256*gk + 128*i + p."""
    return np.ascontiguousarray(
        m.reshape(4, 2, 128, m.shape[1]).transpose(2, 1, 0, 3))


def _shard_inputs(qkv, W_in, b_in, W_out, b_out):
    """Build the 8 per-core input dicts."""
    x = np.asarray(qkv, np.float32)
    W_in = np.asarray(W_in, np.float32)
    b_in = np.asarray(b_in, np.float32)
    W_out = np.asarray(W_out, np.float32)
    scale = np.float32(1.0 / np.sqrt(HD))
    ident = np.eye(128, dtype=np.float16)

    in_maps = []
    for c in range(N_CORES):
        b, g = divmod(c, 2)
        qs = slice(512 * g, 512 * (g + 1))
        ks = slice(1024 + 512 * g, 1024 + 512 * (g + 1))
        vs = slice(2048 + 512 * g, 2048 + 512 * (g + 1))
        bq = b_in[qs].reshape(4, 128).T
        bk = b_in[ks].reshape(4, 128).T
        xT = np.ascontiguousarray(x[b].T)
        in_maps.append({
            "x8": _fold(xT).astype(E8NP),
            "x16": np.ascontiguousarray(
                xT.reshape(8, 128, S).transpose(1, 0, 2)).astype(np.float16),
            "w_qk8": _fold(np.concatenate(
                [W_in[:, qs], W_in[:, ks]], axis=1)).astype(E8NP),
            "w_v16": np.ascontiguousarray(
                W_in[:, vs].reshape(8, 128, 512)
                .transpose(1, 0, 2)).astype(np.float16),
            "b_qk": np.ascontiguousarray(
                np.stack([bq, bk], axis=-1)).astype(np.float32),
            "ident": ident,
            "w_out": np.ascontiguousarray(
                W_out[512 * g:512 * (g + 1)].reshape(4, 128, 1024)
                .transpose(1, 0, 2)).astype(np.float16),
        })
    return in_maps


def kernel(qkv, W_in, b_in, W_out, b_out):
    global _RUNNER
    if _RUNNER is None:
        nc = build_nc()
        _RUNNER = _make_runner(nc, N_CORES)
    in_maps = _shard_inputs(qkv, W_in, b_in, W_out, b_out)
    results = _RUNNER(in_maps)
    b_in = np.asarray(b_in, np.float32)
    W_out_f = np.asarray(W_out, np.float32)
    # V bias folded through the out-projection (softmax rows sum to 1)
    bias = np.asarray(b_out, np.float32) + b_in[2 * D:] @ W_out_f
    out = np.empty((B, S, D), np.float32)
    for b in range(B):
        out[b] = results[2 * b]["out"] + results[2 * b + 1]["out"] + bias
    return out


if __name__ == "__main__":
    rng = np.random.default_rng(0)
    qkv = rng.standard_normal((B, S, D)).astype(np.float32)
    sc = 1.0 / np.sqrt(D)
    W_in = rng.uniform(-sc, sc, (D, 3 * D)).astype(np.float32)
    b_in = rng.uniform(-sc, sc, (3 * D,)).astype(np.float32)
    W_out = rng.uniform(-sc, sc, (D, D)).astype(np.float32)
    b_out = rng.uniform(-sc, sc, (D,)).astype(np.float32)
    got = kernel(qkv, W_in, b_in, W_out, b_out)
    print("kernel ran, output shape", got.shape)
